# revision 30
# baseline (speedup 1.0000x reference)
"""MultiHeadAttention Trainium2 kernel (8 NeuronCores, data-parallel over batch).

Contract: kernel(**inputs) takes the FULL inputs from setup_inputs() and
returns the FULL [8, 512, 1024] output. Internally, batch element c goes to
NeuronCore c (B == n_cores == 8); each core runs the same Bass/Tile program
on its own shard. No collectives needed.

Per-core computation (batch b, S=512, D=1024, H=16, Dk=64):
  QT = (w_q/8)^T-proj of query^T  -> [D, S]  (head h rows h*64..h*64+63)
  KT likewise (unscaled)          -> [D, S]
  V  = natural value proj         -> [S, D]  (stored with a ones column per head)
  per head: scoresT[k,q'] = KT_h-block^T @ QT_h   (q' = reversed query index)
            psum += amt[h]  (host-precomputed fp16 rel-bias + mask additive)
            attnT = exp(psum)                     (ScalarE, reads PSUM)
            ctxT[65,S] = [V_h | 1]^T @ attnT      (row 64 = softmax denominators)
            ctxT_norm = ctxT[0:64] * broadcast(1/denom)
  out_rev[q', e] = ctxT_norm^T-chunks @ w_o^T + b_o ; host un-reverses rows.

All matmuls run in fp32r (single-pass, 4x faster than fp32 LOW_HIGH mode);
matmul operands are pre-rounded to fp32r's 19-bit mantissa on the host (DMA
inputs) or rounded on write by ACT/DVE (on-chip producers), which the BIR
verifier requires. The query-direction reversal makes the relative-position
bias rel_bias[k - q + 511, h] equal rel_bias[k + q', h] — a positive-stride
layout the host can materialize directly.
"""
import numpy as np

import concourse.bass as bass
import concourse.tile as tile
from concourse import bacc, mybir
from concourse.bass_utils import run_bass_kernel_spmd

S = 512
D = 1024
H = 16
DK = 64
N_CORES = 8
NCH = D // 128  # 8 d-model chunks of 128
SB = S // 128   # 4 seq blocks of 128
F32 = mybir.dt.float32
F32R = mybir.dt.float32r
F16 = mybir.dt.float16

MASK_NEG = -30000.0  # large-negative additive mask, fp16-representable

_CACHE = {}


def _build_program():
    nc = bacc.Bacc("TRN2", target_bir_lowering=False, debug=False,
                   num_devices=N_CORES)

    # Per-core DRAM inputs (fp32r ones feed matmuls; host pre-rounds them)
    qT = nc.dram_tensor("qT", [D, S], F32R, kind="ExternalInput").ap()
    kT = nc.dram_tensor("kT", [D, S], F32R, kind="ExternalInput").ap()
    vT = nc.dram_tensor("vT", [D, S], F32R, kind="ExternalInput").ap()
    amt = nc.dram_tensor("amt", [H, S, S], F16, kind="ExternalInput").ap()
    wq = nc.dram_tensor("wq", [D, D], F32R, kind="ExternalInput").ap()
    wk = nc.dram_tensor("wk", [D, D], F32R, kind="ExternalInput").ap()
    wv = nc.dram_tensor("wv", [D, D], F32R, kind="ExternalInput").ap()
    wo = nc.dram_tensor("wo", [D, D], F32R, kind="ExternalInput").ap()
    bq = nc.dram_tensor("bq", [128, NCH], F32, kind="ExternalInput").ap()
    bk = nc.dram_tensor("bk", [128, NCH], F32, kind="ExternalInput").ap()
    bvr = nc.dram_tensor("bvr", [1, D], F32R, kind="ExternalInput").ap()
    bor = nc.dram_tensor("bor", [1, D], F32R, kind="ExternalInput").ap()
    out = nc.dram_tensor("out", [S, D], F32, kind="ExternalOutput").ap()

    # DRAM views for chunked DMA
    qT3 = qT.rearrange("(c p) s -> p c s", p=128)     # [128, 8, 512]
    kT3 = kT.rearrange("(c p) s -> p c s", p=128)
    vT3 = vT.rearrange("(c p) s -> p c s", p=128)
    amt4 = amt.rearrange("h (kb p) q -> h p kb q", p=128)  # [16, 128, 4, 512]
    wq3 = wq.rearrange("(c p) e -> c p e", p=128)     # [8, 128, 1024]
    wk3 = wk.rearrange("(c p) e -> c p e", p=128)
    wv3 = wv.rearrange("(c p) e -> c p e", p=128)
    wo3 = wo.rearrange("(c p) e -> c p e", p=128)
    out3 = out.rearrange("(sb p) e -> sb p e", p=128)  # [4, 128, 1024]

    from contextlib import ExitStack

    with tile.TileContext(nc) as tc, ExitStack() as ctx:
        singles = ctx.enter_context(tc.tile_pool(name="singles", bufs=1))
        wpool = ctx.enter_context(tc.tile_pool(name="wpool", bufs=16))
        rcpool = ctx.enter_context(tc.tile_pool(name="rcpool", bufs=2))
        rc1pool = ctx.enter_context(tc.tile_pool(name="rc1pool", bufs=1))
        ps_proj = ctx.enter_context(tc.tile_pool(name="ps_proj", bufs=2, space="PSUM"))
        ps_sc = ctx.enter_context(tc.tile_pool(name="ps_sc", bufs=3, space="PSUM"))
        ps_ctx = ctx.enter_context(tc.tile_pool(name="ps_ctx", bufs=2, space="PSUM"))
        ps_r = ctx.enter_context(tc.tile_pool(name="ps_r", bufs=1, space="PSUM"))
        vt_ctx = ExitStack()
        vtpool = vt_ctx.enter_context(tc.tile_pool(name="vtpool", bufs=1))

        # small constants first so the warm-up matmuls can start immediately
        bq_sb = singles.tile([128, NCH], F32, tag="bq")
        bk_sb = singles.tile([128, NCH], F32, tag="bk")
        nc.sync.dma_start(out=bq_sb, in_=bq)
        nc.sync.dma_start(out=bk_sb, in_=bk)
        bvr_sb = singles.tile([1, D], F32R, tag="bvr")
        bor_sb = singles.tile([1, D], F32R, tag="bor")
        nc.sync.dma_start(out=bvr_sb, in_=bvr)
        nc.sync.dma_start(out=bor_sb, in_=bor)
        # memset can't target fp32r; stage in fp32 and round via ACT copy
        ones_f32 = singles.tile([1, 128], F32, tag="ones_f32")
        nc.vector.memset(ones_f32, 1.0)
        ones_sb = singles.tile([1, 128], F32R, tag="ones")
        nc.scalar.copy(ones_sb, ones_f32)
        ones_col = singles.tile([128, H, 1], F32, tag="ones_col")
        nc.vector.memset(ones_col, 1.0)

        # HAM warm-up: ~5us of throwaway matmuls while the input DMAs stream,
        # so the PE clock-gate is at 8/8 by the time real work is ready.
        # Operands are built on-chip so no DMA gates the first matmul.
        for _ in range(40):
            pd = ps_proj.tile([128, 512], F32, tag="proj")
            nc.tensor.matmul(pd[:, :128], lhsT=ones_sb[:, :128], rhs=ones_sb,
                             start=True, stop=True)

        # bulk loads, emitted in consumption order (sync queue is FIFO)
        vT_sb = vtpool.tile([128, NCH, S], F32R, tag="vT")
        nc.sync.dma_start(out=vT_sb, in_=vT3)
        qT_sb = singles.tile([128, NCH, S], F32R, tag="qT")
        kT_sb = singles.tile([128, NCH, S], F32R, tag="kT")

        # big persistent activations
        QT_sb = singles.tile([128, NCH, S], F32R, tag="QT")
        KT_sb = singles.tile([128, NCH, S], F32R, tag="KT")
        # V with a ones column appended per head: [128, sb, 16*65]
        V_sb = singles.tile([128, SB, H * (DK + 1)], F32R, tag="V")
        ctxT_sb = singles.tile([128, NCH, S], F32R, tag="ctxT")

        # ---- V projection: V[s, e] = vT^T @ wvT + b_v ----
        wv_sb = []
        for dc in range(NCH):
            t = wpool.tile([128, D], F32R, tag="w")
            nc.sync.dma_start(out=t, in_=wv3[dc])
            wv_sb.append(t)
        nc.sync.dma_start(out=qT_sb, in_=qT3)
        nc.sync.dma_start(out=kT_sb, in_=kT3)
        for sb in range(SB):
            # set ones columns for this s-block
            v_heads = V_sb[:, sb, :].rearrange("p (h c) -> p h c", c=DK + 1)
            nc.scalar.copy(v_heads[:, :, DK:DK + 1], ones_col)
            for eh in range(2):
                pv = ps_proj.tile([128, 512], F32, tag="proj")
                for dc in range(NCH):
                    nc.tensor.matmul(
                        pv,
                        lhsT=vT_sb[:, dc, sb * 128:(sb + 1) * 128],
                        rhs=wv_sb[dc][:, eh * 512:(eh + 1) * 512],
                        start=(dc == 0), stop=False,
                    )
                nc.tensor.matmul(
                    pv, lhsT=ones_sb[:, :128],
                    rhs=bvr_sb[:, eh * 512:(eh + 1) * 512],
                    start=False, stop=True,
                )
                nc.scalar.copy(
                    v_heads[:, 8 * eh:8 * eh + 8, 0:DK],
                    pv.rearrange("p (h d) -> p h d", d=DK),
                )

        # vT no longer needed; free its SBUF for the pools below
        vt_ctx.close()
        amtpool = ctx.enter_context(tc.tile_pool(name="amtpool", bufs=2))
        attnpool = ctx.enter_context(tc.tile_pool(name="attnpool", bufs=9))
        outpool = ctx.enter_context(tc.tile_pool(name="outpool", bufs=2))

        # ---- interleaved Q/K projection chunks + attention heads ----
        wq_sb = []
        wk_sb = []
        for dc in range(NCH):
            t = wpool.tile([128, D], F32R, tag="w")
            nc.sync.dma_start(out=t, in_=wq3[dc])
            wq_sb.append(t)
        for dc in range(NCH):
            t = wpool.tile([128, D], F32R, tag="w")
            nc.sync.dma_start(out=t, in_=wk3[dc])
            wk_sb.append(t)
        # wo prefetch: emitted here so it sits early on the sync queue; its
        # slot-wait resolves as soon as the wv tiles retire after V-proj
        wo_sb = []
        for ch in range(NCH):
            t = wpool.tile([128, D], F32R, tag="w")
            nc.sync.dma_start(out=t, in_=wo3[ch])
            wo_sb.append(t)

        # Software pipeline over heads: head h's context matmuls are emitted
        # one head later, so PE fills the DVE-add -> ACT-exp latency of head
        # h with head h+1's scores (and the next chunk's projections) and
        # never idles long enough for HAM to re-throttle.
        def emit_scores(h):
            i, p0 = h // 2, (h % 2) * 64
            amt_h = amtpool.tile([128, SB, S], F16, tag="amt")
            # GpSimd (SWDGE) queue: keeps amt streams off the sync queue so
            # weight prefetch (esp. wo) isn't stuck behind them, and off the
            # busy compute engines' FIFOs (GpSimd is otherwise nearly idle)
            nc.gpsimd.dma_start(out=amt_h, in_=amt4[h])
            QT_h = QT_sb[p0:p0 + 64, i, :]
            attn_tiles = []
            for kb in range(SB):
                ps = ps_sc.tile([128, 512], F32, tag="sc")
                nc.tensor.matmul(
                    ps, lhsT=KT_sb[p0:p0 + 64, i, kb * 128:(kb + 1) * 128],
                    rhs=QT_h, start=True, stop=True,
                )
                nc.vector.tensor_add(ps, ps, amt_h[:, kb, :])
                at = attnpool.tile([128, 512], F32R, tag="attn")
                nc.scalar.activation(at, ps, mybir.ActivationFunctionType.Exp)
                attn_tiles.append(at)
            return attn_tiles

        def emit_ctx(h, attn_tiles):
            i, p0 = h // 2, (h % 2) * 64
            pc = ps_ctx.tile([DK + 1, 512], F32, tag="ctx")
            for kb in range(SB):
                nc.tensor.matmul(
                    pc, lhsT=V_sb[:, kb, h * 65:(h + 1) * 65],
                    rhs=attn_tiles[kb], start=(kb == 0), stop=(kb == SB - 1),
                )
            # custom-DVE reciprocal can't read PSUM on HW; stage sums in SBUF
            sums_sb = rcpool.tile([1, 512], F32, tag="recip")
            nc.scalar.copy(sums_sb, pc[DK:DK + 1, :])
            recip_f32 = rcpool.tile([1, 512], F32, tag="recip")
            nc.vector.reciprocal_approx_fast(out=recip_f32, in_=sums_sb)
            recip = rc1pool.tile([1, 512], F32R, tag="recip_r")
            nc.scalar.copy(recip, recip_f32)
            # broadcast 1/denom across 64 partitions via a K=1 matmul
            # (GpSimd's queue is reserved for the amt DMA stream)
            pr = ps_r.tile([64, 512], F32, tag="r")
            nc.tensor.matmul(pr, lhsT=ones_sb[:, :64], rhs=recip,
                             start=True, stop=True)
            r_sb = rc1pool.tile([64, 512], F32, tag="rbc")
            nc.scalar.copy(r_sb, pr)
            nc.vector.tensor_mul(ctxT_sb[p0:p0 + 64, i, :], pc[0:DK, :], r_sb)

        pending = None  # (head, attn_tiles) awaiting its context matmuls
        for i in range(NCH):  # e-chunk i covers heads 2i, 2i+1
            pq = ps_proj.tile([128, 512], F32, tag="proj")
            for dc in range(NCH):
                nc.tensor.matmul(
                    pq, lhsT=wq_sb[dc][:, i * 128:(i + 1) * 128],
                    rhs=qT_sb[:, dc, :],
                    start=(dc == 0), stop=(dc == NCH - 1),
                )
            nc.scalar.add(QT_sb[:, i, :], pq, bq_sb[:, i:i + 1])
            pk = ps_proj.tile([128, 512], F32, tag="proj")
            for dc in range(NCH):
                nc.tensor.matmul(
                    pk, lhsT=wk_sb[dc][:, i * 128:(i + 1) * 128],
                    rhs=kT_sb[:, dc, :],
                    start=(dc == 0), stop=(dc == NCH - 1),
                )
            nc.scalar.add(KT_sb[:, i, :], pk, bk_sb[:, i:i + 1])

            for sub in range(2):
                h = 2 * i + sub
                tiles = emit_scores(h)
                if pending is not None:
                    emit_ctx(*pending)
                pending = (h, tiles)
        emit_ctx(*pending)

        # ---- output projection: out_rev[q', e] = ctxT^T @ woT + b_o ----
        for sb in range(SB):
            for eh in range(2):
                po = ps_proj.tile([128, 512], F32, tag="proj")
                for ch in range(NCH):
                    nc.tensor.matmul(
                        po, lhsT=ctxT_sb[:, ch, sb * 128:(sb + 1) * 128],
                        rhs=wo_sb[ch][:, eh * 512:(eh + 1) * 512],
                        start=(ch == 0), stop=False,
                    )
                nc.tensor.matmul(
                    po, lhsT=ones_sb[:, :128],
                    rhs=bor_sb[:, eh * 512:(eh + 1) * 512],
                    start=False, stop=True,
                )
                osb = outpool.tile([128, 512], F32, tag="out")
                nc.scalar.copy(osb, po)
                nc.sync.dma_start(
                    out=out3[sb, :, eh * 512:(eh + 1) * 512], in_=osb)

    nc.compile()
    return nc


def _round_f32r(a):
    """Round-to-nearest to fp32r's 19-bit (1+8+13... wait — explicit 13-bit)
    mantissa so the PE's fp32r truncation is lossless on these operands."""
    b = np.ascontiguousarray(a, np.float32).view(np.uint32)
    return ((b + 0x1000) & np.uint32(0xFFFFE000)).view(np.float32)


def _prep_inputs(query, key, value, mask, w_q, b_q, w_k, b_k, w_v, b_v,
                 w_o, b_o, rel_bias):
    query = np.asarray(query, np.float32)
    key = np.asarray(key, np.float32)
    value = np.asarray(value, np.float32)
    mask = np.asarray(mask)
    w_q = np.asarray(w_q, np.float32)
    w_k = np.asarray(w_k, np.float32)
    w_v = np.asarray(w_v, np.float32)
    w_o = np.asarray(w_o, np.float32)
    b_q = np.asarray(b_q, np.float32)
    b_k = np.asarray(b_k, np.float32)
    b_v = np.asarray(b_v, np.float32)
    b_o = np.asarray(b_o, np.float32)
    rel_bias = np.asarray(rel_bias, np.float32)

    shared = {
        "wq": _round_f32r(w_q.T / 8.0),
        "wk": _round_f32r(w_k.T),
        "wv": _round_f32r(w_v.T),
        "wo": _round_f32r(w_o.T),
        "bq": np.ascontiguousarray((b_q / 8.0).reshape(NCH, 128).T),
        "bk": np.ascontiguousarray(b_k.reshape(NCH, 128).T),
        "bvr": _round_f32r(b_v.reshape(1, D)),
        "bor": _round_f32r(b_o.reshape(1, D)),
    }

    # biasT_rev[h, k, q'] = rel_bias[k + q', h]
    idx = np.arange(S)[:, None] + np.arange(S)[None, :]  # [k, q'] in [0, 1022]
    bias_t = rel_bias[idx]                 # [S, S, H]
    bias_t = np.ascontiguousarray(bias_t.transpose(2, 0, 1))  # [H, k, q']

    in_maps = []
    for c in range(N_CORES):
        # maskT_rev[k, q'] additive: mask[c, 0, 511-q', k] == 0 -> MASK_NEG
        m = mask[c, 0][::-1, :].T          # [k, q'] values in {0, 1}
        madd = np.where(m == 0, np.float32(MASK_NEG), np.float32(0.0))
        amt = (bias_t + madd[None]).astype(np.float16)
        im = dict(shared)
        im["qT"] = _round_f32r(query[c].T[:, ::-1])
        im["kT"] = _round_f32r(key[c].T)
        im["vT"] = _round_f32r(value[c].T)
        im["amt"] = np.ascontiguousarray(amt)
        in_maps.append(im)
    return in_maps


def kernel(query, key, value, mask, w_q, b_q, w_k, b_k, w_v, b_v, w_o, b_o,
           rel_bias, _run_opts=None):
    if "nc" not in _CACHE:
        _CACHE["nc"] = _build_program()
    nc = _CACHE["nc"]
    in_maps = _prep_inputs(query, key, value, mask, w_q, b_q, w_k, b_k,
                           w_v, b_v, w_o, b_o, rel_bias)
    opts = _run_opts or {}
    res = run_bass_kernel_spmd(nc, in_maps, list(range(N_CORES)), **opts)
    out = np.stack([res.results[c]["out"][::-1, :] for c in range(N_CORES)])
    if _run_opts is not None:
        _CACHE["last_result"] = res
    return out.astype(np.float32)
